# revision 62
# baseline (speedup 1.0000x reference)
"""BackflowMLP Trainium2 kernel.

Strategy: 8-way tensor-parallel over the 65536-dim output of the big
Dense (512x512 @ 512x65536); each core computes the full 512-batch
trunk MLP (replicated, small) and an 8192-feature shard of the output
matmul in fp8 e4m3 DoubleRow mode.

The schedule targets the three binding resources in order:
1. The PSUM->fp8 epilogue: GPSIMD cannot read PSUM on TRN2, so casting
   the 4.2MB/core of matmul output is an ACT+DVE two-engine stream
   (~17.8us) and paces the endgame. PSUM runs as 4x[128,1024] 2-bank
   tiles so the PE->cast loop stays 4 deep; casts are greedy-balanced
   across ACT (1038ns) and DVE (1192ns) per 2-tile group.
2. The serialized DMA resource (~24.6us of traffic at 360GB/s): Wout
   loads stream as interleaved 0.5MB fp8 pieces with the last two
   pieces emitted between the first stores, and stores go out on the
   SP queue sized [1,1,1,2x13,1,1,1] (in 2-tile units) so the store
   stream starts right as the upfront loads drain and the final chain
   carries a short transfer.
3. The trunk critical chain feeding the stream: s is {0,1} so gelu(s)=
   gelu(1)*s folds into the skip projection, per-feature biases fold
   into the weights as bias/64 (each s row has exactly 64 ones), xT
   ships as fp8 (exact), Wcb0 ships fp8 at half scale read twice via
   stride-0 broadcast DoubleRow APs (no zero padding), W1 runs
   fp8 DoubleRow pre-scaled by SW1, and the block-1 skip re-accumulates
   s@Wcb01 in PSUM (no x1 materialization, no identity weights). The
   trunk gelus are bias-free [128,1024] ops; the big loop opens with
   pair-0-only head matmuls while the pair-1 half of g2 quantizes.
A PE warmup chain rides out the pstate ramp and a dummy gelu preloads
the ACT function table. Output ships as scaled fp8 e4m3; bias add,
/sqrt(2), complex assembly, gather, logdet and logsumexp run on host.
"""

import numpy as np
import ml_dtypes

N_ORB, N_A, N_B, N_DETS = 64, 32, 32, 16
K = 32
H0 = H1 = 512
IN_DIM = 128
OUT_DIM = N_DETS * N_ORB * K            # 32768
OUT2 = 2 * OUT_DIM                      # 65536
B = 512
NCORES = 8
TP = 8
BSH = B                                 # full batch on every core
OSH = OUT2 // TP                        # 8192 output features per core
OT = OSH // 128                         # 64 output tiles per core
GROUP = 4                               # output tiles per store DMA
SW = 65536.0 * 1.25                     # Wout fp8 scale
SW1 = 64.0 * 1.05                       # W1 fp8 scale (off the pow2 grid)
CO = 2.0 ** -5                          # PSUM -> fp8 store scale

_CACHE = {}


def _build_nc(nwc=1):
    import concourse.mybir as mybir
    import concourse.tile as tile
    from concourse import bacc

    fp32 = mybir.dt.float32
    bf16 = mybir.dt.bfloat16
    f8 = mybir.dt.float8e4
    GELU = mybir.ActivationFunctionType.Gelu_apprx_tanh
    COPY = mybir.ActivationFunctionType.Copy
    DR = mybir.MatmulPerfMode.DoubleRow

    nc = bacc.Bacc(
        "TRN2", target_bir_lowering=False, debug=False, num_devices=NCORES
    )

    # packed small loads. s is exactly {0,1} so xT ships as fp8 (exact).
    # Every s row has exactly 64 ones, so per-feature biases fold into the
    # contraction weights as bias/64 and the trunk gelus run bias-free.
    XT8 = nc.declare_dram_parameter("XT8", [IN_DIM, BSH], f8, isOutput=False)
    # Wcb0 (and Wcb01 when b1 != 0) as fp8 at half scale: both DoubleRow
    # halves read the same rows via a stride-0 broadcast AP (w/2 + w/2 = w),
    # and the moving xT broadcasts across the pair dim likewise, so block-0
    # and skip matmuls run at the 0.5 cycles/col DoubleRow rate with no
    # zero-padding bytes shipped.
    W0Q = nc.declare_dram_parameter("W0Q", [IN_DIM, nwc, H0], f8,
                                    isOutput=False)
    # W1 * SW1 in fp8 DoubleRow layout: W1Q[p, pair, half, f] holds
    # W1[pair*256 + half*128 + p, f] (K = 512 contraction rows).
    W1Q = nc.declare_dram_parameter("W1Q", [128, 2, 2, H1], f8, isOutput=False)
    # Wout fp8, scaled by SW, DoubleRow layout: W8<pair>[p, half, f] holds
    # Wout row pair*256 + half*128 + p.
    W8a = nc.declare_dram_parameter("W8a", [128, 2, OSH], f8, isOutput=False)
    W8b = nc.declare_dram_parameter("W8b", [128, 2, OSH], f8, isOutput=False)
    # y[p, ot, b] = (g2 @ Wout)[b, ot*128+p] * SW*CO
    yT = nc.declare_dram_parameter("yT", [128, OT, BSH], f8, isOutput=True)

    with tile.TileContext(nc) as tc:
        with (
            tc.tile_pool(name="wp", bufs=1) as wp,
            tc.tile_pool(name="ap_", bufs=1) as ap_,
            tc.tile_pool(name="ppsy", bufs=4, space="PSUM") as ppsy,
        ):
            # ---- persistent loads (all on the SP queue, in order) ----
            xwb = wp.tile([128, nwc, H0], f8, tag="xwb")
            nc.sync.dma_start(xwb[:], W0Q[:])
            xt8 = wp.tile([128, BSH], f8, tag="xt8")
            nc.sync.dma_start(xt8[:], XT8[:])

            def xt_ap():
                return xt8[:].unsqueeze(1).broadcast_to([128, 2, BSH])

            def wc_ap(mt):
                return xwb[:, 0, mt * 128:(mt + 1) * 128].unsqueeze(
                    1).broadcast_to([128, 2, 128])

            def wc1_ap(mt):
                return xwb[:, nwc - 1, mt * 128:(mt + 1) * 128].unsqueeze(
                    1).broadcast_to([128, 2, 128])

            w1q = wp.tile([128, 2, 2, H1], f8, tag="w1q")
            nc.sync.dma_start(w1q[:], W1Q[:])

            # Wout fp8: 2 pairs x 4 feature-pieces, interleaved by pair so
            # output tiles unlock in 16-tile waves right as the trunk ends.
            w8p = [
                wp.tile([128, 2, OSH], f8, tag=f"w8_{p}", name=f"w8_{p}")
                for p in range(2)
            ]
            QPIECE = OSH // 4
            # pieces 0-2 of each pair load up front; piece 3 of each pair is
            # emitted mid-loop, interleaved between the first stores, so the
            # serialized DMA resource processes loads and stores in the order
            # compute needs them (stores would otherwise all queue behind
            # every load and the tail would go DMA-idle).
            for piece in range(3):
                for pair, W8x in ((0, W8a), (1, W8b)):
                    nc.sync.dma_start(
                        w8p[pair][:, :, piece * QPIECE:(piece + 1) * QPIECE],
                        W8x[:, :, piece * QPIECE:(piece + 1) * QPIECE],
                    )
            def load_tail_piece(pair):
                if pair > 1:
                    return
                W8x = (W8a, W8b)[pair]
                nc.sync.dma_start(
                    w8p[pair][:, :, 3 * QPIECE:], W8x[:, :, 3 * QPIECE:]
                )

            # ---- PE warmup: keep the PE continuously busy on zeros so the
            # pstate ramp (low->mid->full at 3us) completes before real work;
            # the dummy gelu triggers the ACT function-table load at t~1us
            # instead of on the trunk critical path.
            dum = wp.tile([128, 64], bf16, tag="dum")
            nc.vector.memset(dum[:], 0.0)
            dumg = ap_.tile([128, 8], bf16, tag="dumg")
            nc.scalar.activation(dumg[:], dum[:, :8], GELU)
            ps_d = ppsy.tile([128, 2 * BSH], fp32, tag="psy")
            for _ in range(24):
                nc.tensor.matmul(ps_d[:64, :64], dum[:, :64], dum[:],
                                 start=True, stop=True)

            # ---- trunk block 0: x1 = s@Wcb0 (bias folded into Wcb0),
            # g1 = gelu(x1) written straight to fp8 in DoubleRow pair layout.
            # Two 128-feature chunks share one 2-bank PSUM tile so a single
            # bias-free [128,1024] ACT op produces a whole g8 pair.
            g8 = [
                ap_.tile([128, 2, BSH], f8, tag=f"g1_{p}", name=f"g1_{p}")
                for p in range(2)
            ]
            b0ps = []
            for pair in range(2):
                r_ps = ppsy.tile([128, 2 * BSH], fp32, tag="psy",
                                 name=f"b0ps_{pair}")
                b0ps.append(r_ps)
                for hh in range(2):
                    mt = 2 * pair + hh
                    nc.tensor.matmul(r_ps[:, hh * BSH:(hh + 1) * BSH],
                                     wc_ap(mt), xt_ap(),
                                     start=True, stop=True, perf_mode=DR)
            for pair in range(2):
                nc.scalar.activation(g8[pair][:], b0ps[pair][:],
                                     GELU, scale=1.0 / SW1)

            # ---- trunk block 1: x2 = g1@W1 + x1 + b1, all in PSUM at scale
            # SW1: the skip re-accumulates s@Wcb01 (b1 folded), g1@W1 runs
            # fp8 DoubleRow; one bias-free gelu2 per pair writes g2 fp8
            # DoubleRow pairs for the big loop.
            grhs = [
                ap_.tile([128, 2, BSH], f8, tag=f"g8_{p}", name=f"g8_{p}")
                for p in range(2)
            ]
            h_ps = [ppsy.tile([128, 2 * BSH], fp32, tag="psy",
                              name=f"h_ps_{i}")
                    for i in range(2)]

            def b1_half(mt):
                return h_ps[mt // 2][:, (mt % 2) * BSH:(mt % 2 + 1) * BSH]
            # all skips, then all kp0, then all kp1: the single wait on the
            # pair-1 gelu1 is absorbed once instead of once per chunk
            for mt in range(4):
                nc.tensor.matmul(b1_half(mt), wc1_ap(mt), xt_ap(),
                                 start=True, stop=False, perf_mode=DR)
            for kp in range(2):
                for mt in range(4):
                    nc.tensor.matmul(
                        b1_half(mt),
                        w1q[:, kp, :, mt * 128:(mt + 1) * 128],
                        g8[kp][:],
                        start=False, stop=(kp == 1), perf_mode=DR,
                    )
            for pair in range(2):
                nc.scalar.activation(grhs[pair][:], h_ps[pair][:],
                                     GELU, scale=1.0 / SW1)

            # ---- big output matmul: fp8 DoubleRow, K=2x(256+256) ----
            # PSUM tiles hold TWO 128-feature output tiles (2 banks); one
            # ACT or DVE [128,1024] op casts both (GPSIMD cannot read PSUM,
            # so the epilogue is an ACT+DVE two-engine stream and is the
            # pacing resource of the whole endgame). Stores go out on the SP
            # queue, interleaved with the final W8 pieces; the first and last
            # store groups are 2-tile so the store stream starts right as the
            # upfront loads drain and the final chain carries a short
            # transfer. HEAD groups run their pair-0 contraction first so
            # the PE has work while the pair-1 half of g2 quantizes.
            stage = None
            NOT2 = OT // 2
            HEAD = 3
            y_head = []
            for ot2 in range(HEAD):
                y_ps = ppsy.tile([128, 2 * BSH], fp32, tag="psy",
                                 name=f"yh_{ot2}")
                y_head.append(y_ps)
                for h in range(2):
                    ot = 2 * ot2 + h
                    nc.tensor.matmul(y_ps[:, h * BSH:(h + 1) * BSH],
                                     w8p[0][:, :, ot * 128:(ot + 1) * 128],
                                     grhs[0][:],
                                     start=True, stop=False, perf_mode=DR)
            # cast engine schedule per 2-tile group: 0=ACT, 1=DVE. Balanced
            # by cast work only (the trunk gelus finish before the first y
            # group exists, so both engines are free from T0 on); the final
            # group is reserved for ACT, which drains its plan first.
            cast_plan = []
            cnt = {0: 32, 1: 32}
            rates = {0: 1038.0, 1: 1192.0}
            load = {0: 1038.0, 1: 0.0}   # reserve the final group for ACT
            for _ in range(NOT2 - 1):
                e = min((k for k in cnt if cnt[k] > 0),
                        key=lambda k: load[k] + rates[k])
                cast_plan.append(e)
                cnt[e] -= 1
                load[e] += rates[e]
            cast_plan.append(0)   # final group on ACT (drains first)

            # store groups in 2-tile units
            sgroups2 = [1, 1, 1] + [2] * 13 + [1, 1, 1]
            assert sum(sgroups2) == NOT2
            sg_i = 0
            sg_fill = 0
            for ot2 in range(NOT2):
                if ot2 < HEAD:
                    y_ps = y_head[ot2]
                    for h in range(2):
                        ot = 2 * ot2 + h
                        nc.tensor.matmul(
                            y_ps[:, h * BSH:(h + 1) * BSH],
                            w8p[1][:, :, ot * 128:(ot + 1) * 128],
                            grhs[1][:],
                            start=False, stop=True, perf_mode=DR)
                else:
                    y_ps = ppsy.tile([128, 2 * BSH], fp32, tag="psy",
                                     name=f"y_{ot2}")
                    for h in range(2):
                        ot = 2 * ot2 + h
                        dst_ps = y_ps[:, h * BSH:(h + 1) * BSH]
                        for pair in range(2):
                            nc.tensor.matmul(
                                dst_ps,
                                w8p[pair][:, :, ot * 128:(ot + 1) * 128],
                                grhs[pair][:],
                                start=(pair == 0), stop=(pair == 1),
                                perf_mode=DR)
                gsz = sgroups2[sg_i]
                if sg_fill == 0:
                    stage = ap_.tile([128, 2 * gsz * BSH], f8, tag="stage",
                                     name="stage", bufs=12,
                                     padded_shape=[128, GROUP * BSH])
                dst = stage[:, 2 * sg_fill * BSH:2 * (sg_fill + 1) * BSH]
                if cast_plan[ot2] == -1:
                    nc.scalar.activation(dst[:, :BSH], y_ps[:, :BSH],
                                         COPY, scale=CO)
                    nc.vector.tensor_scalar_mul(dst[:, BSH:], y_ps[:, BSH:],
                                                CO)
                elif cast_plan[ot2] == 0:
                    nc.scalar.activation(dst, y_ps[:], COPY, scale=CO)
                else:
                    nc.vector.tensor_scalar_mul(dst, y_ps[:], CO)
                sg_fill += 1
                if sg_fill == gsz:
                    ot = 2 * ot2 + 1
                    # overlap the final stores' issue chains across queues
                    issuer = nc.scalar if sg_i == len(sgroups2) - 2 else nc.sync
                    issuer.dma_start(
                        yT[:, ot + 1 - 2 * gsz:ot + 1, :], stage[:]
                    )
                    # slot the final W8 pieces between the first stores
                    if sg_i < 2:
                        load_tail_piece(sg_i)
                    sg_i += 1
                    sg_fill = 0
    nc.compile()
    return nc


def _get_nc(nwc=1):
    key = ("nc", nwc)
    if key not in _CACHE:
        _CACHE[key] = _build_nc(nwc)
        _CACHE["nc"] = _CACHE[key]   # for tooling that grabs the last build
    return _CACHE[key]


def kernel(**inputs):
    import os
    import time
    os.environ["BASS_NEVER_TRACE"] = "1"   # NTFF hook module absent in this build
    from concourse import bass_utils

    s = np.asarray(inputs["s"])
    W0 = np.asarray(inputs["W0"], np.float32)
    b0 = np.asarray(inputs["b0"], np.float32)
    Ws0 = np.asarray(inputs["Ws0"], np.float32)
    bs0 = np.asarray(inputs["bs0"], np.float32)
    W1 = np.asarray(inputs["W1"], np.float32)
    b1 = np.asarray(inputs["b1"], np.float32)
    Wout = np.asarray(inputs["Wout"], np.float32)
    bout = np.asarray(inputs["bout"], np.float32)
    M = np.asarray(inputs["M"])
    log_c = np.asarray(inputs["log_c"])

    bf = ml_dtypes.bfloat16
    e4 = ml_dtypes.float8_e4m3
    xT_full = np.ascontiguousarray(s.astype(np.float32).T)              # [128, 512]
    # s is {0,1} exactly, so gelu(s) = gelu(1)*s: fold the gelu branch of
    # block 0 into the skip projection (tanh-approx gelu at x=1, fp64).
    # Every s row also has exactly 64 ones, so per-feature biases fold into
    # the weights as bias/64: block 0 uses Wcb0 (bias0 folded), the block-1
    # skip uses Wcb01 (bias0+b1 folded; same tensor when b1 == 0).
    g1c = 0.5 * (1.0 + np.tanh(np.sqrt(2.0 / np.pi) * (1.0 + 0.044715)))
    Wcb = Ws0.astype(np.float64) + g1c * W0.astype(np.float64)
    bias0 = (b0 + bs0).astype(np.float64)
    bias1 = b1.astype(np.float64)
    Wcb0 = (Wcb + bias0[None, :] / 64.0) * SW1
    nwc = 1 if not bias1.any() else 2
    XT8h = np.ascontiguousarray(xT_full).astype(e4)   # {0,1}: exact in fp8
    blocks = [Wcb0]
    if nwc == 2:
        Wcb01 = (Wcb + (bias0 + bias1)[None, :] / 64.0) * SW1
        blocks.append(Wcb01)
    W0Qh = np.zeros((IN_DIM, nwc, H0), e4)
    for bi, blk in enumerate(blocks):
        W0Qh[:, bi, :] = np.clip(blk / 2.0, -240.0, 240.0).astype(e4)
    # W1 * SW1 -> fp8 DoubleRow K-pair layout [p, pair, half, f]
    W1q = np.clip(W1.astype(np.float64) * SW1, -240.0, 240.0).astype(e4)
    W1q = np.ascontiguousarray(
        W1q.reshape(2, 2, 128, H1).transpose(2, 0, 1, 3)
    )

    # Wout -> scaled fp8 in DoubleRow layout [p, half, f] per pair
    Wq = np.clip(Wout * SW, -240.0, 240.0).astype(e4)
    Wq = Wq.reshape(2, 2, 128, OUT2)      # [pair, half, p, f]
    in_maps = []
    for i in range(NCORES):
        cols = slice(i * OSH, (i + 1) * OSH)
        in_maps.append({
            "XT8": XT8h,
            "W0Q": W0Qh,
            "W1Q": W1q,
            "W8a": np.ascontiguousarray(Wq[0, :, :, cols].transpose(1, 0, 2)),
            "W8b": np.ascontiguousarray(Wq[1, :, :, cols].transpose(1, 0, 2)),
        })

    nc = _get_nc(nwc)
    t0 = time.perf_counter()
    res = bass_utils.run_bass_kernel_spmd(nc, in_maps, core_ids=list(range(NCORES)))
    _CACHE["last_exec_ns"] = res.exec_time_ns
    _CACHE["last_wall_ns"] = (time.perf_counter() - t0) * 1e9

    y = np.empty((B, OUT2), np.float32)
    inv = 1.0 / (SW * CO)
    for i in range(NCORES):
        arr = res.results[i]["yT"].astype(np.float32) * inv    # [128, OT, BSH]
        y[:, i * OSH:(i + 1) * OSH] = arr.transpose(1, 0, 2).reshape(OSH, BSH).T

    # ---- host tail: bias, complex assembly, gather, logdet, logsumexp ----
    isq = 1.0 / np.sqrt(2.0)
    re = y[:, :OUT_DIM] * isq + bout[:OUT_DIM] * isq
    im = y[:, OUT_DIM:] * isq + bout[OUT_DIM:] * isq
    delta = (re + 1j * im).astype(np.complex64).reshape(B, N_DETS, N_ORB, K)
    M_eff = M[None].astype(np.complex64) + delta

    rows_a = np.argsort(1 - s[:, :N_ORB], axis=-1, kind="stable")[:, :N_A]
    rows_b = np.argsort(1 - s[:, N_ORB:], axis=-1, kind="stable")[:, :N_B]
    ia = np.broadcast_to(rows_a[:, None, :, None], (B, N_DETS, N_A, K))
    ib = np.broadcast_to(rows_b[:, None, :, None], (B, N_DETS, N_B, K))
    A_a = np.take_along_axis(M_eff, ia, axis=2)[..., :N_A]
    A_b = np.take_along_axis(M_eff, ib, axis=2)[..., :N_B]

    sign_a, lad_a = np.linalg.slogdet(A_a.astype(np.complex128))
    sign_b, lad_b = np.linalg.slogdet(A_b.astype(np.complex128))
    log_dets = np.log(sign_a) + lad_a + np.log(sign_b) + lad_b + log_c[None]

    m = np.max(log_dets.real, axis=1, keepdims=True)
    out = m[:, 0] + np.log(np.sum(np.exp(log_dets - m), axis=1))
    return out.astype(np.complex64)
